# revision 23
# baseline (speedup 1.0000x reference)
"""TransformerConv MixerBlock (x + TransformerConv(x, edge_index)) on 8 trn2 NeuronCores.

Strategy (v5): permute+bin-pack nodes into 128-node tiles balanced by
in-degree (49 tiles/core, KU=16 edge chunks of 128 per tile). Host streams,
per edge chunk, x of the edge's SOURCE node transposed (x_edgeT: [c, e]) and
q of the edge's DESTINATION node transposed (q_edgeT: [c, e]) - both pure
permutations/small host prep, so the device never does a random gather.

Device per chunk j (128 edges), [channel, edge] orientation:
  kT_j = wkT^T-matmul(rhs=x_edgeT_j)        -> PSUM [c,e]   (PE)
  v_j  = matmul(lhsT=x_edgeT_j, rhs=wvT)    -> PSUM [e,c]   (PE)
  qk   = q_edgeT_j * kT_j                   -> SBUF f16     (DVE, 2x via
                                               Act PSUM->SBUF kT copy)
  alpha= matmul(lhsT=qk, rhs=head_indicator)-> PSUM [e,4]   (PE - replaces
                                               the DVE per-head reduction)
  a    = exp(alpha)                         -> SBUF f16     (Act)
  X    = v_j * a[head-bcast] | a            -> SBUF f16     (DVE/gp)
  psS += matmul(lhsT=ohT_j, rhs=X)          -> PSUM [n,132] (PE)
where ohT (the scatter one-hot, [e, n]) is GENERATED ON DEVICE from compact
dst-local indices with one is_equal op per tile in an (n,KU)-interleaved
layout that keeps all operands packed (DVE 2x eligible) - this removes both
51MB/core one-hot HBM streams of v4. Tail: normalize by the accumulated
denominator columns, add skip+residual, DMA out.
"""
import sys, os, types, math, heapq
sys.path.insert(0, '/opt/trn_rl_repo')
import numpy as np

P = 128
D = 128
H = 4
DH = 32
NCORES = 8
NBG = 4          # chunks per PSUM group

_prog_cache = {}


def _ensure_hooks():
    """Best-effort shim of antenv.axon_hooks so trace=True profiling works."""
    try:
        import antenv
        if 'antenv.axon_hooks' not in sys.modules:
            mod = types.ModuleType('antenv.axon_hooks')
            state = {'hook': None}
            mod.set_axon_ntff_profile_hook = lambda h: state.__setitem__('hook', h)
            mod.get_axon_ntff_profile_hook = lambda: state['hook']
            sys.modules['antenv.axon_hooks'] = mod
            antenv.axon_hooks = mod
            from trn_agent_boot.trn_boot import _ntff_profile_via_ctypes
            hook = _ntff_profile_via_ctypes('/opt/axon/libaxon_pjrt.so')
            if hook is not None:
                mod.set_axon_ntff_profile_hook(hook)
    except Exception:
        pass
    try:
        import concourse.bass_utils as bass_utils
        bass_utils.upload_artifacts = lambda tmpdir: tmpdir
    except Exception:
        pass


def _prep(x, edge_index, Wq, bq, Wk, bk, Wv, bv, Wskip, bskip):
    N = x.shape[0]
    E = edge_index.shape[1]
    TPC = (N + NCORES * P - 1) // (NCORES * P)
    NT = NCORES * TPC

    src = np.asarray(edge_index[0], dtype=np.int64)
    dst = np.asarray(edge_index[1], dtype=np.int64)
    deg = np.bincount(dst, minlength=N)

    # --- bin-pack nodes into NT tiles of <=P nodes, balancing degree sums ---
    order = np.argsort(-deg, kind='stable')
    heap = [(0, t) for t in range(NT)]
    heapq.heapify(heap)
    counts = np.zeros(NT, dtype=np.int64)
    tile_deg = np.zeros(NT, dtype=np.int64)
    node_slot = np.empty(N, dtype=np.int64)
    for n in order:
        while True:
            dsum, t = heapq.heappop(heap)
            if counts[t] < P:
                break
        node_slot[n] = t * P + counts[t]
        counts[t] += 1
        tile_deg[t] += deg[n]
        if counts[t] < P:
            heapq.heappush(heap, (dsum + int(deg[n]), t))
    KU = max(1, int((tile_deg.max() + P - 1) // P))

    # --- permuted node features and host-side q = x @ Wq^T / sqrt(DH) ---
    s = 1.0 / math.sqrt(DH)
    xf = np.asarray(x, dtype=np.float32)
    q_full = (xf @ np.asarray(Wq, dtype=np.float32).T) * s
    x_perm = np.zeros((NT * P, D), dtype=np.float16)
    x_perm[node_slot] = xf.astype(np.float16)
    q_perm = np.zeros((NT * P, D), dtype=np.float16)
    q_perm[node_slot] = q_full.astype(np.float16)
    x_permT = x_perm.T.copy()

    # --- per-tile edge lists (sorted by src slot for locality) ---
    src_slot = node_slot[src]
    dst_slot = node_slot[dst]
    et = dst_slot // P
    key = et * (1 << 32) + src_slot
    eorder = np.argsort(key, kind='stable')
    et_s = et[eorder]
    src_s = src_slot[eorder]
    dst_s = dst_slot[eorder]
    dloc_s = dst_s - et_s * P

    ecnt = np.bincount(et, minlength=NT)
    eoff = np.zeros(NT + 1, dtype=np.int64)
    np.cumsum(ecnt, out=eoff[1:])
    pos = np.arange(E) - eoff[et_s]

    # padded per-tile edge arrays: slot (tile, chunk j, part e) = edge j*128+e
    src_pad = np.zeros(NT * KU * P, dtype=np.int64)
    dst_pad = np.zeros(NT * KU * P, dtype=np.int64)
    dl_pad = np.full(NT * KU * P, 255, dtype=np.int64)
    flat = et_s * (KU * P) + pos
    src_pad[flat] = src_s
    dst_pad[flat] = dst_s
    dl_pad[flat] = dloc_s

    # x/q in edge order, transposed, tile-major so each tile's stream is one
    # contiguous HBM region: [NT, D, KU*P]
    x_edgeT = np.ascontiguousarray(
        x_perm[src_pad].T.reshape(D, NT, KU * P).transpose(1, 0, 2))
    q_edgeT = np.ascontiguousarray(
        q_perm[dst_pad].T.reshape(D, NT, KU * P).transpose(1, 0, 2))

    # compact dst-local indices: [P(e), NT*KU], f16 (0..127 or 255 pad)
    dl_all = dl_pad.reshape(NT, KU, P).transpose(2, 0, 1).astype(np.float16)

    # device consts
    iota_rep = np.tile(np.repeat(np.arange(P), KU).astype(np.float16),
                       (P, 1))                      # [P, P*KU], val[n*KU+j]=n
    blockind = (np.arange(D)[:, None] // DH
                == np.arange(H)[None, :]).astype(np.float16)   # [D, H]

    wkT = np.asarray(Wk, dtype=np.float32).T.astype(np.float16).copy()
    wvT = np.asarray(Wv, dtype=np.float32).T.astype(np.float16).copy()
    wsT = np.asarray(Wskip, dtype=np.float32).T.astype(np.float16).copy()
    for b in (bq, bk, bv, bskip):
        assert np.abs(np.asarray(b)).max() == 0.0, "nonzero biases not supported"

    in_maps = []
    for c in range(NCORES):
        t0, t1 = c * TPC, (c + 1) * TPC
        in_maps.append({
            "x_loc": x_perm[t0 * P:t1 * P].copy(),
            "x_locT": x_permT[:, t0 * P:t1 * P].copy(),
            "x_edgeT": x_edgeT[t0:t1].copy(),
            "q_edgeT": q_edgeT[t0:t1].copy(),
            "dl": dl_all[:, t0:t1, :].reshape(P, TPC * KU).copy(),
            "iota_rep": iota_rep, "blockind": blockind,
            "wkT": wkT, "wvT": wvT, "wsT": wsT,
        })
    return dict(N=N, E=E, TPC=TPC, NT=NT, KU=KU,
                node_slot=node_slot, in_maps=in_maps)


def _build(TPC, NT, KU):
    import concourse.bass as bass
    import concourse.bacc as bacc
    import concourse.mybir as mybir
    import concourse.tile as tile

    f16 = mybir.dt.float16
    f32 = mybir.dt.float32
    MUL = mybir.AluOpType.mult
    ADD = mybir.AluOpType.add
    ISEQ = mybir.AluOpType.is_equal
    EXP = mybir.ActivationFunctionType.Exp
    COPY = mybir.ActivationFunctionType.Copy
    AXX = mybir.AxisListType.X

    nc = bacc.Bacc("TRN2", target_bir_lowering=False, debug=False)
    x_loc = nc.dram_tensor("x_loc", [TPC * P, D], f16, kind="ExternalInput")
    x_locT = nc.dram_tensor("x_locT", [D, TPC * P], f16, kind="ExternalInput")
    x_edgeT = nc.dram_tensor("x_edgeT", [TPC, D, KU * P], f16,
                             kind="ExternalInput")
    q_edgeT = nc.dram_tensor("q_edgeT", [TPC, D, KU * P], f16,
                             kind="ExternalInput")
    dl_in = nc.dram_tensor("dl", [P, TPC * KU], f16, kind="ExternalInput")
    iota_in = nc.dram_tensor("iota_rep", [P, P * KU], f16, kind="ExternalInput")
    bi_in = nc.dram_tensor("blockind", [D, H], f16, kind="ExternalInput")
    wkT = nc.dram_tensor("wkT", [D, D], f16, kind="ExternalInput")
    wvT = nc.dram_tensor("wvT", [D, D], f16, kind="ExternalInput")
    wsT = nc.dram_tensor("wsT", [D, D], f16, kind="ExternalInput")
    out_t = nc.dram_tensor("out", [TPC * P, D], f32, kind="ExternalOutput")

    NB = 4
    groups = [(g * NBG, min(NBG, KU - g * NBG)) for g in range((KU + NBG - 1) // NBG)]

    with tile.TileContext(nc) as tc:
        with (
            tc.tile_pool(name="const", bufs=1) as cp,
            tc.tile_pool(name="sbuf", bufs=3) as sb,
            tc.tile_pool(name="big", bufs=3) as bigp,
            tc.tile_pool(name="psK", bufs=3, space="PSUM") as pk,
            tc.tile_pool(name="psV", bufs=3, space="PSUM") as pv,
            tc.tile_pool(name="psS", bufs=2, space="PSUM") as ps,
        ):
            wk_sb = cp.tile([D, D], f16, tag="wk")
            wv_sb = cp.tile([D, D], f16, tag="wv")
            ws_sb = cp.tile([D, D], f16, tag="ws")
            bi_sb = cp.tile([D, H], f16, tag="bi")
            iota_sb = cp.tile([P, P * KU], f16, tag="iota")
            dl_sb = cp.tile([P, TPC * KU], f16, tag="dl")
            s_loc = cp.tile([P, TPC * D], f16, tag="sloc")
            nc.sync.dma_start(out=wk_sb[:], in_=wkT[:])
            nc.sync.dma_start(out=wv_sb[:], in_=wvT[:])
            nc.sync.dma_start(out=ws_sb[:], in_=wsT[:])
            nc.sync.dma_start(out=bi_sb[:], in_=bi_in[:])
            nc.sync.dma_start(out=iota_sb[:], in_=iota_in[:])
            nc.sync.dma_start(out=dl_sb[:], in_=dl_in[:])

            # ---------------- edge phase (local skip interleaved) ----------------
            for u in range(TPC):
                if u % NB == 0:
                    # local skip block: s_loc[u:u+lb] = x@Wskip^T + x
                    lb = min(NB, TPC - u)
                    xTl = sb.tile([P, NB * P], f16, tag="xT")
                    nc.sync.dma_start(
                        out=xTl[:, :lb * P], in_=x_locT[:, u * P:(u + lb) * P])
                    pq = pk.tile([P, NB * P], f32, tag="k")
                    for b in range(lb):
                        nc.tensor.matmul(pq[:, b * P:(b + 1) * P],
                                         lhsT=xTl[:, b * P:(b + 1) * P],
                                         rhs=ws_sb[:], start=True, stop=True)
                    xl = sb.tile([P, NB, P], f16, tag="xl")
                    nc.sync.dma_start(
                        out=xl[:, :lb, :],
                        in_=x_loc[u * P:(u + lb) * P, :].rearrange(
                            "(b p) c -> p b c", p=P))
                    nc.vector.tensor_tensor(
                        out=s_loc[:, u * D:(u + lb) * D].rearrange(
                            "p (b c) -> p b c", c=P),
                        in0=pq[:, :lb * P].rearrange("p (b c) -> p b c", c=P),
                        in1=xl[:, :lb, :], op=ADD)
                xeT = bigp.tile([P, KU * P], f16, tag="xeT")
                nc.sync.dma_start(out=xeT[:], in_=x_edgeT[u, :, :])
                qeT = bigp.tile([P, KU * P], f16, tag="qeT")
                nc.sync.dma_start(out=qeT[:], in_=q_edgeT[u, :, :])
                # scatter one-hot, interleaved layout [e, (n,KU)]:
                # ohT[e, n*KU+j] = (dl[e, u*KU+j] == n)
                ohT = bigp.tile([P, P * KU], f16, tag="ohT")
                nc.vector.tensor_tensor(
                    out=ohT[:].rearrange("p (n j) -> p n j", j=KU),
                    in0=iota_sb[:].rearrange("p (n j) -> p n j", j=KU),
                    in1=dl_sb[:, None, u * KU:(u + 1) * KU].to_broadcast(
                        [P, P, KU]),
                    op=ISEQ)
                ohT3 = ohT[:].rearrange("p (n j) -> p n j", j=KU)

                psS = ps.tile([P, 132], f32, tag="acc")
                for gi, (j0, gsz) in enumerate(groups):
                    psK = pk.tile([P, NB * P], f32, tag="k")
                    psV = pv.tile([P, NBG, P], f32, tag="v")
                    for cc in range(gsz):
                        j = j0 + cc
                        nc.tensor.matmul(
                            psK[:, cc * P:(cc + 1) * P],
                            lhsT=wk_sb[:], rhs=xeT[:, j * P:(j + 1) * P],
                            start=True, stop=True)
                    for cc in range(gsz):
                        j = j0 + cc
                        nc.tensor.matmul(
                            psV[:, cc, :],
                            lhsT=xeT[:, j * P:(j + 1) * P], rhs=wv_sb[:],
                            start=True, stop=True)
                    # kT PSUM->SBUF f16 on Act so the qk multiply runs 2x
                    ksb = sb.tile([P, NBG * P], f16, tag="ksb")
                    nc.scalar.activation(out=ksb[:, :gsz * P],
                                         in_=psK[:, :gsz * P], func=COPY)
                    qk = sb.tile([P, NBG * P], f16, tag="qk")
                    qk_eng = nc.vector if (gi == 3 and u % 2 == 0) else nc.gpsimd
                    qk_eng.tensor_tensor(
                        out=qk[:, :gsz * P],
                        in0=qeT[:, j0 * P:j0 * P + gsz * P],
                        in1=ksb[:, :gsz * P], op=MUL)
                    # per-head reduction on PE: alpha[e,h] = sum_c qk[c,e]*bi[c,h]
                    # reuses the head of the psK tile (k already copied to ksb)
                    psA = psK[:, 0:NBG * H]
                    for cc in range(gsz):
                        nc.tensor.matmul(
                            psA[:, cc * H:(cc + 1) * H],
                            lhsT=qk[:, cc * P:(cc + 1) * P], rhs=bi_sb[:],
                            start=True, stop=True)
                    X = sb.tile([P, NBG, 132], f16, tag="X")
                    nc.scalar.activation(
                        out=X[:, :gsz, 128:132],
                        in_=psA[:, :gsz * H].rearrange("p (a h) -> p a h", h=H),
                        func=EXP)
                    nc.vector.tensor_tensor(
                        out=X[:, :gsz, 0:128].rearrange(
                            "p a (h e) -> p a h e", e=DH),
                        in0=psV[:, :gsz, :].rearrange(
                            "p a (h e) -> p a h e", e=DH),
                        in1=X[:, :gsz, 128:132, None].to_broadcast(
                            [P, gsz, H, DH]),
                        op=MUL)
                    for cc in range(gsz):
                        j = j0 + cc
                        nc.tensor.matmul(
                            psS[:, 0:132],
                            lhsT=ohT3[:, :, j],
                            rhs=X[:, cc, 0:132],
                            start=(j == 0), stop=(j == KU - 1))

                dn = sb.tile([P, H], f32, tag="dn")
                nc.vector.tensor_scalar(out=dn[:], in0=psS[:, 128:132],
                                        scalar1=1e-16, scalar2=None, op0=ADD)
                rc = sb.tile([P, H], f32, tag="rc")
                nc.vector.reciprocal(out=rc[:], in_=dn[:])
                ot = sb.tile([P, D], f32, tag="ot")
                for h in range(H):
                    nc.scalar.activation(
                        out=ot[:, h * DH:(h + 1) * DH],
                        in_=psS[:, h * DH:(h + 1) * DH],
                        func=COPY, scale=rc[:, h:h + 1])
                of = sb.tile([P, D], f32, tag="of")
                nc.gpsimd.tensor_tensor(
                    out=of[:], in0=ot[:], in1=s_loc[:, u * D:(u + 1) * D], op=ADD)
                nc.sync.dma_start(out=out_t[u * P:(u + 1) * P, :], in_=of[:])

    nc.finalize()
    return nc


def _run(inputs, trace=False):
    _ensure_hooks()
    from concourse.bass_utils import run_bass_kernel_spmd

    meta = _prep(**inputs)
    key = (meta['TPC'], meta['NT'], meta['KU'])
    if key not in _prog_cache:
        _prog_cache[key] = _build(*key)
    nc = _prog_cache[key]
    res = run_bass_kernel_spmd(nc, meta['in_maps'],
                               core_ids=list(range(NCORES)), trace=trace)
    outs = [res.results[c]["out"] for c in range(NCORES)]
    out_perm = np.concatenate(outs, axis=0)
    out = out_perm[meta['node_slot']].astype(np.float32)
    return out, res


def kernel(**inputs) -> np.ndarray:
    out, _ = _run(inputs, trace=False)
    return out


# revision 25
# speedup vs baseline: 1.0806x; 1.0806x over previous
"""TransformerConv MixerBlock (x + TransformerConv(x, edge_index)) on 8 trn2 NeuronCores.

Strategy (v5): permute+bin-pack nodes into 128-node tiles balanced by
in-degree (49 tiles/core, KU=16 edge chunks of 128 per tile). Host streams,
per edge chunk, x of the edge's SOURCE node transposed (x_edgeT: [c, e]) and
q of the edge's DESTINATION node transposed (q_edgeT: [c, e]) - both pure
permutations/small host prep, so the device never does a random gather.

Device per chunk j (128 edges), [channel, edge] orientation:
  kT_j = wkT^T-matmul(rhs=x_edgeT_j)        -> PSUM [c,e]   (PE)
  v_j  = matmul(lhsT=x_edgeT_j, rhs=wvT)    -> PSUM [e,c]   (PE)
  qk   = q_edgeT_j * kT_j                   -> SBUF f16     (DVE, 2x via
                                               Act PSUM->SBUF kT copy)
  alpha= matmul(lhsT=qk, rhs=head_indicator)-> PSUM [e,4]   (PE - replaces
                                               the DVE per-head reduction)
  a    = exp(alpha)                         -> SBUF f16     (Act)
  X    = v_j * a[head-bcast] | a            -> SBUF f16     (DVE/gp)
  psS += matmul(lhsT=ohT_j, rhs=X)          -> PSUM [n,132] (PE)
where ohT (the scatter one-hot, [e, n]) is GENERATED ON DEVICE from compact
dst-local indices with one is_equal op per tile in an (n,KU)-interleaved
layout that keeps all operands packed (DVE 2x eligible) - this removes both
51MB/core one-hot HBM streams of v4. Tail: normalize by the accumulated
denominator columns, add skip+residual, DMA out.
"""
import sys, os, types, math, heapq
sys.path.insert(0, '/opt/trn_rl_repo')
import numpy as np

P = 128
D = 128
H = 4
DH = 32
NCORES = 8
NBG = 4          # chunks per PSUM group

_prog_cache = {}


def _ensure_hooks():
    """Best-effort shim of antenv.axon_hooks so trace=True profiling works."""
    try:
        import antenv
        if 'antenv.axon_hooks' not in sys.modules:
            mod = types.ModuleType('antenv.axon_hooks')
            state = {'hook': None}
            mod.set_axon_ntff_profile_hook = lambda h: state.__setitem__('hook', h)
            mod.get_axon_ntff_profile_hook = lambda: state['hook']
            sys.modules['antenv.axon_hooks'] = mod
            antenv.axon_hooks = mod
            from trn_agent_boot.trn_boot import _ntff_profile_via_ctypes
            hook = _ntff_profile_via_ctypes('/opt/axon/libaxon_pjrt.so')
            if hook is not None:
                mod.set_axon_ntff_profile_hook(hook)
    except Exception:
        pass
    try:
        import concourse.bass_utils as bass_utils
        bass_utils.upload_artifacts = lambda tmpdir: tmpdir
    except Exception:
        pass


def _prep(x, edge_index, Wq, bq, Wk, bk, Wv, bv, Wskip, bskip):
    N = x.shape[0]
    E = edge_index.shape[1]
    TPC = (N + NCORES * P - 1) // (NCORES * P)
    NT = NCORES * TPC

    src = np.asarray(edge_index[0], dtype=np.int64)
    dst = np.asarray(edge_index[1], dtype=np.int64)
    deg = np.bincount(dst, minlength=N)

    # --- bin-pack nodes into NT tiles of <=P nodes, balancing degree sums ---
    order = np.argsort(-deg, kind='stable')
    heap = [(0, t) for t in range(NT)]
    heapq.heapify(heap)
    counts = np.zeros(NT, dtype=np.int64)
    tile_deg = np.zeros(NT, dtype=np.int64)
    node_slot = np.empty(N, dtype=np.int64)
    for n in order:
        while True:
            dsum, t = heapq.heappop(heap)
            if counts[t] < P:
                break
        node_slot[n] = t * P + counts[t]
        counts[t] += 1
        tile_deg[t] += deg[n]
        if counts[t] < P:
            heapq.heappush(heap, (dsum + int(deg[n]), t))
    KU = max(1, int((tile_deg.max() + P - 1) // P))

    # --- permuted node features and host-side q = x @ Wq^T / sqrt(DH) ---
    s = 1.0 / math.sqrt(DH)
    xf = np.asarray(x, dtype=np.float32)
    q_full = (xf @ np.asarray(Wq, dtype=np.float32).T) * s
    x_perm = np.zeros((NT * P, D), dtype=np.float16)
    x_perm[node_slot] = xf.astype(np.float16)
    q_perm = np.zeros((NT * P, D), dtype=np.float16)
    q_perm[node_slot] = q_full.astype(np.float16)
    x_permT = x_perm.T.copy()

    # --- per-tile edge lists (sorted by src slot for locality) ---
    src_slot = node_slot[src]
    dst_slot = node_slot[dst]
    et = dst_slot // P
    key = et * (1 << 32) + src_slot
    eorder = np.argsort(key, kind='stable')
    et_s = et[eorder]
    src_s = src_slot[eorder]
    dst_s = dst_slot[eorder]
    dloc_s = dst_s - et_s * P

    ecnt = np.bincount(et, minlength=NT)
    eoff = np.zeros(NT + 1, dtype=np.int64)
    np.cumsum(ecnt, out=eoff[1:])
    pos = np.arange(E) - eoff[et_s]

    # padded per-tile edge arrays: slot (tile, chunk j, part e) = edge j*128+e
    src_pad = np.zeros(NT * KU * P, dtype=np.int64)
    dst_pad = np.zeros(NT * KU * P, dtype=np.int64)
    dl_pad = np.full(NT * KU * P, 255, dtype=np.int64)
    flat = et_s * (KU * P) + pos
    src_pad[flat] = src_s
    dst_pad[flat] = dst_s
    dl_pad[flat] = dloc_s

    # x/q in edge order, transposed, tile-major so each tile's stream is one
    # contiguous HBM region: [NT, D, KU*P]
    x_edgeT = np.ascontiguousarray(
        x_perm[src_pad].T.reshape(D, NT, KU * P).transpose(1, 0, 2))
    q_edgeT = np.ascontiguousarray(
        q_perm[dst_pad].T.reshape(D, NT, KU * P).transpose(1, 0, 2))

    # compact dst-local indices: [P(e), NT*KU], f16 (0..127 or 255 pad)
    dl_all = dl_pad.reshape(NT, KU, P).transpose(2, 0, 1).astype(np.float16)

    # device consts
    iota_rep = np.tile(np.repeat(np.arange(P), KU).astype(np.float16),
                       (P, 1))                      # [P, P*KU], val[n*KU+j]=n
    blockind = (np.arange(D)[:, None] // DH
                == np.arange(H)[None, :]).astype(np.float16)   # [D, H]

    wkT = np.asarray(Wk, dtype=np.float32).T.astype(np.float16).copy()
    wvT = np.asarray(Wv, dtype=np.float32).T.astype(np.float16).copy()
    wsT = np.asarray(Wskip, dtype=np.float32).T.astype(np.float16).copy()
    for b in (bq, bk, bv, bskip):
        assert np.abs(np.asarray(b)).max() == 0.0, "nonzero biases not supported"

    in_maps = []
    for c in range(NCORES):
        t0, t1 = c * TPC, (c + 1) * TPC
        in_maps.append({
            "x_loc": x_perm[t0 * P:t1 * P].copy(),
            "x_locT": x_permT[:, t0 * P:t1 * P].copy(),
            "x_edgeT": x_edgeT[t0:t1].copy(),
            "q_edgeT": q_edgeT[t0:t1].copy(),
            "dl": dl_all[:, t0:t1, :].reshape(P, TPC * KU).copy(),
            "iota_rep": iota_rep, "blockind": blockind,
            "wkT": wkT, "wvT": wvT, "wsT": wsT,
        })
    return dict(N=N, E=E, TPC=TPC, NT=NT, KU=KU,
                node_slot=node_slot, in_maps=in_maps)


def _build(TPC, NT, KU):
    import concourse.bass as bass
    import concourse.bacc as bacc
    import concourse.mybir as mybir
    import concourse.tile as tile

    f16 = mybir.dt.float16
    f32 = mybir.dt.float32
    MUL = mybir.AluOpType.mult
    ADD = mybir.AluOpType.add
    ISEQ = mybir.AluOpType.is_equal
    EXP = mybir.ActivationFunctionType.Exp
    COPY = mybir.ActivationFunctionType.Copy
    AXX = mybir.AxisListType.X

    nc = bacc.Bacc("TRN2", target_bir_lowering=False, debug=False)
    x_loc = nc.dram_tensor("x_loc", [TPC * P, D], f16, kind="ExternalInput")
    x_locT = nc.dram_tensor("x_locT", [D, TPC * P], f16, kind="ExternalInput")
    x_edgeT = nc.dram_tensor("x_edgeT", [TPC, D, KU * P], f16,
                             kind="ExternalInput")
    q_edgeT = nc.dram_tensor("q_edgeT", [TPC, D, KU * P], f16,
                             kind="ExternalInput")
    dl_in = nc.dram_tensor("dl", [P, TPC * KU], f16, kind="ExternalInput")
    iota_in = nc.dram_tensor("iota_rep", [P, P * KU], f16, kind="ExternalInput")
    bi_in = nc.dram_tensor("blockind", [D, H], f16, kind="ExternalInput")
    wkT = nc.dram_tensor("wkT", [D, D], f16, kind="ExternalInput")
    wvT = nc.dram_tensor("wvT", [D, D], f16, kind="ExternalInput")
    wsT = nc.dram_tensor("wsT", [D, D], f16, kind="ExternalInput")
    out_t = nc.dram_tensor("out", [TPC * P, D], f32, kind="ExternalOutput")

    NB = 4
    groups = [(g * NBG, min(NBG, KU - g * NBG)) for g in range((KU + NBG - 1) // NBG)]

    with tile.TileContext(nc) as tc:
        with (
            tc.tile_pool(name="const", bufs=1) as cp,
            tc.tile_pool(name="sbuf", bufs=3) as sb,
            tc.tile_pool(name="big", bufs=3) as bigp,
            tc.tile_pool(name="psK", bufs=2, space="PSUM") as pk,
            tc.tile_pool(name="psV", bufs=2, space="PSUM") as pv,
            tc.tile_pool(name="psAl", bufs=2, space="PSUM") as pal,
            tc.tile_pool(name="psS", bufs=2, space="PSUM") as ps,
        ):
            wk_sb = cp.tile([D, D], f16, tag="wk")
            wv_sb = cp.tile([D, D], f16, tag="wv")
            ws_sb = cp.tile([D, D], f16, tag="ws")
            bi_sb = cp.tile([D, H], f16, tag="bi")
            iota_sb = cp.tile([P, P * KU], f16, tag="iota")
            dl_sb = cp.tile([P, TPC * KU], f16, tag="dl")
            s_loc = cp.tile([P, TPC * D], f16, tag="sloc")
            nc.sync.dma_start(out=wk_sb[:], in_=wkT[:])
            nc.sync.dma_start(out=wv_sb[:], in_=wvT[:])
            nc.sync.dma_start(out=ws_sb[:], in_=wsT[:])
            nc.sync.dma_start(out=bi_sb[:], in_=bi_in[:])
            nc.sync.dma_start(out=iota_sb[:], in_=iota_in[:])
            nc.sync.dma_start(out=dl_sb[:], in_=dl_in[:])

            # ---------------- edge phase (local skip interleaved) ----------------
            for u in range(TPC):
                if u % NB == 0:
                    # local skip block: s_loc[u:u+lb] = x@Wskip^T + x
                    lb = min(NB, TPC - u)
                    xTl = sb.tile([P, NB * P], f16, tag="xT")
                    nc.sync.dma_start(
                        out=xTl[:, :lb * P], in_=x_locT[:, u * P:(u + lb) * P])
                    pq = pk.tile([P, NB * P], f32, tag="k")
                    for b in range(lb):
                        nc.tensor.matmul(pq[:, b * P:(b + 1) * P],
                                         lhsT=xTl[:, b * P:(b + 1) * P],
                                         rhs=ws_sb[:], start=True, stop=True)
                    xl = sb.tile([P, NB, P], f16, tag="xl")
                    nc.sync.dma_start(
                        out=xl[:, :lb, :],
                        in_=x_loc[u * P:(u + lb) * P, :].rearrange(
                            "(b p) c -> p b c", p=P))
                    nc.vector.tensor_tensor(
                        out=s_loc[:, u * D:(u + lb) * D].rearrange(
                            "p (b c) -> p b c", c=P),
                        in0=pq[:, :lb * P].rearrange("p (b c) -> p b c", c=P),
                        in1=xl[:, :lb, :], op=ADD)
                xeT = bigp.tile([P, KU * P], f16, tag="xeT")
                nc.sync.dma_start(out=xeT[:], in_=x_edgeT[u, :, :])
                qeT = bigp.tile([P, KU * P], f16, tag="qeT")
                nc.sync.dma_start(out=qeT[:], in_=q_edgeT[u, :, :])
                # scatter one-hot, interleaved layout [e, (n,KU)]:
                # ohT[e, n*KU+j] = (dl[e, u*KU+j] == n)
                ohT = bigp.tile([P, P * KU], f16, tag="ohT")
                nc.vector.tensor_tensor(
                    out=ohT[:].rearrange("p (n j) -> p n j", j=KU),
                    in0=iota_sb[:].rearrange("p (n j) -> p n j", j=KU),
                    in1=dl_sb[:, None, u * KU:(u + 1) * KU].to_broadcast(
                        [P, P, KU]),
                    op=ISEQ)
                ohT3 = ohT[:].rearrange("p (n j) -> p n j", j=KU)

                psS = ps.tile([P, 132], f32, tag="acc")
                for gi, (j0, gsz) in enumerate(groups):
                    psK = pk.tile([P, NB * P], f32, tag="k")
                    psV = pv.tile([P, NBG, P], f32, tag="v")
                    for cc in range(gsz):
                        j = j0 + cc
                        nc.tensor.matmul(
                            psK[:, cc * P:(cc + 1) * P],
                            lhsT=wk_sb[:], rhs=xeT[:, j * P:(j + 1) * P],
                            start=True, stop=True)
                    for cc in range(gsz):
                        j = j0 + cc
                        nc.tensor.matmul(
                            psV[:, cc, :],
                            lhsT=xeT[:, j * P:(j + 1) * P], rhs=wv_sb[:],
                            start=True, stop=True)
                    # kT PSUM->SBUF f16 on Act so the qk multiply runs 2x
                    ksb = sb.tile([P, NBG * P], f16, tag="ksb")
                    nc.scalar.activation(out=ksb[:, :gsz * P],
                                         in_=psK[:, :gsz * P], func=COPY)
                    qk = sb.tile([P, NBG * P], f16, tag="qk")
                    qk_eng = nc.vector if (gi == 3 and u % 2 == 0) else nc.gpsimd
                    qk_eng.tensor_tensor(
                        out=qk[:, :gsz * P],
                        in0=qeT[:, j0 * P:j0 * P + gsz * P],
                        in1=ksb[:, :gsz * P], op=MUL)
                    # per-head reduction on PE: alpha[e,h] = sum_c qk[c,e]*bi[c,h]
                    psA = pal.tile([P, NBG * H], f32, tag="al")
                    for cc in range(gsz):
                        nc.tensor.matmul(
                            psA[:, cc * H:(cc + 1) * H],
                            lhsT=qk[:, cc * P:(cc + 1) * P], rhs=bi_sb[:],
                            start=True, stop=True)
                    X = sb.tile([P, NBG, 132], f16, tag="X")
                    nc.scalar.activation(
                        out=X[:, :gsz, 128:132],
                        in_=psA[:, :gsz * H].rearrange("p (a h) -> p a h", h=H),
                        func=EXP)
                    nc.vector.tensor_tensor(
                        out=X[:, :gsz, 0:128].rearrange(
                            "p a (h e) -> p a h e", e=DH),
                        in0=psV[:, :gsz, :].rearrange(
                            "p a (h e) -> p a h e", e=DH),
                        in1=X[:, :gsz, 128:132, None].to_broadcast(
                            [P, gsz, H, DH]),
                        op=MUL)
                    for cc in range(gsz):
                        j = j0 + cc
                        nc.tensor.matmul(
                            psS[:, 0:132],
                            lhsT=ohT3[:, :, j],
                            rhs=X[:, cc, 0:132],
                            start=(j == 0), stop=(j == KU - 1))

                dn = sb.tile([P, H], f32, tag="dn")
                nc.vector.tensor_scalar(out=dn[:], in0=psS[:, 128:132],
                                        scalar1=1e-16, scalar2=None, op0=ADD)
                rc = sb.tile([P, H], f32, tag="rc")
                nc.vector.reciprocal(out=rc[:], in_=dn[:])
                ot = sb.tile([P, D], f32, tag="ot")
                for h in range(H):
                    nc.scalar.activation(
                        out=ot[:, h * DH:(h + 1) * DH],
                        in_=psS[:, h * DH:(h + 1) * DH],
                        func=COPY, scale=rc[:, h:h + 1])
                of = sb.tile([P, D], f32, tag="of")
                nc.gpsimd.tensor_tensor(
                    out=of[:], in0=ot[:], in1=s_loc[:, u * D:(u + 1) * D], op=ADD)
                nc.sync.dma_start(out=out_t[u * P:(u + 1) * P, :], in_=of[:])

    nc.finalize()
    return nc


def _run(inputs, trace=False):
    _ensure_hooks()
    from concourse.bass_utils import run_bass_kernel_spmd

    meta = _prep(**inputs)
    key = (meta['TPC'], meta['NT'], meta['KU'])
    if key not in _prog_cache:
        _prog_cache[key] = _build(*key)
    nc = _prog_cache[key]
    res = run_bass_kernel_spmd(nc, meta['in_maps'],
                               core_ids=list(range(NCORES)), trace=trace)
    outs = [res.results[c]["out"] for c in range(NCORES)]
    out_perm = np.concatenate(outs, axis=0)
    out = out_perm[meta['node_slot']].astype(np.float32)
    return out, res


def kernel(**inputs) -> np.ndarray:
    out, _ = _run(inputs, trace=False)
    return out


# revision 26
# speedup vs baseline: 1.1477x; 1.0621x over previous
"""TransformerConv MixerBlock (x + TransformerConv(x, edge_index)) on 8 trn2 NeuronCores.

Strategy (v5): permute+bin-pack nodes into 128-node tiles balanced by
in-degree (49 tiles/core, KU=16 edge chunks of 128 per tile). Host streams,
per edge chunk, x of the edge's SOURCE node transposed (x_edgeT: [c, e]) and
q of the edge's DESTINATION node transposed (q_edgeT: [c, e]) - both pure
permutations/small host prep, so the device never does a random gather.

Device per chunk j (128 edges), [channel, edge] orientation:
  kT_j = wkT^T-matmul(rhs=x_edgeT_j)        -> PSUM [c,e]   (PE)
  v_j  = matmul(lhsT=x_edgeT_j, rhs=wvT)    -> PSUM [e,c]   (PE)
  qk   = q_edgeT_j * kT_j                   -> SBUF f16     (DVE, 2x via
                                               Act PSUM->SBUF kT copy)
  alpha= matmul(lhsT=qk, rhs=head_indicator)-> PSUM [e,4]   (PE - replaces
                                               the DVE per-head reduction)
  a    = exp(alpha)                         -> SBUF f16     (Act)
  X    = v_j * a[head-bcast] | a            -> SBUF f16     (DVE/gp)
  psS += matmul(lhsT=ohT_j, rhs=X)          -> PSUM [n,132] (PE)
where ohT (the scatter one-hot, [e, n]) is GENERATED ON DEVICE from compact
dst-local indices with one is_equal op per tile in an (n,KU)-interleaved
layout that keeps all operands packed (DVE 2x eligible) - this removes both
51MB/core one-hot HBM streams of v4. Tail: normalize by the accumulated
denominator columns, add skip+residual, DMA out.
"""
import sys, os, types, math, heapq
sys.path.insert(0, '/opt/trn_rl_repo')
import numpy as np

P = 128
D = 128
H = 4
DH = 32
NCORES = 8
NBG = 4          # chunks per PSUM group

_prog_cache = {}


def _ensure_hooks():
    """Best-effort shim of antenv.axon_hooks so trace=True profiling works."""
    try:
        import antenv
        if 'antenv.axon_hooks' not in sys.modules:
            mod = types.ModuleType('antenv.axon_hooks')
            state = {'hook': None}
            mod.set_axon_ntff_profile_hook = lambda h: state.__setitem__('hook', h)
            mod.get_axon_ntff_profile_hook = lambda: state['hook']
            sys.modules['antenv.axon_hooks'] = mod
            antenv.axon_hooks = mod
            from trn_agent_boot.trn_boot import _ntff_profile_via_ctypes
            hook = _ntff_profile_via_ctypes('/opt/axon/libaxon_pjrt.so')
            if hook is not None:
                mod.set_axon_ntff_profile_hook(hook)
    except Exception:
        pass
    try:
        import concourse.bass_utils as bass_utils
        bass_utils.upload_artifacts = lambda tmpdir: tmpdir
    except Exception:
        pass


def _prep(x, edge_index, Wq, bq, Wk, bk, Wv, bv, Wskip, bskip):
    N = x.shape[0]
    E = edge_index.shape[1]
    TPC = (N + NCORES * P - 1) // (NCORES * P)
    NT = NCORES * TPC

    src = np.asarray(edge_index[0], dtype=np.int64)
    dst = np.asarray(edge_index[1], dtype=np.int64)
    deg = np.bincount(dst, minlength=N)

    # --- bin-pack nodes into NT tiles of <=P nodes, balancing degree sums ---
    order = np.argsort(-deg, kind='stable')
    heap = [(0, t) for t in range(NT)]
    heapq.heapify(heap)
    counts = np.zeros(NT, dtype=np.int64)
    tile_deg = np.zeros(NT, dtype=np.int64)
    node_slot = np.empty(N, dtype=np.int64)
    for n in order:
        while True:
            dsum, t = heapq.heappop(heap)
            if counts[t] < P:
                break
        node_slot[n] = t * P + counts[t]
        counts[t] += 1
        tile_deg[t] += deg[n]
        if counts[t] < P:
            heapq.heappush(heap, (dsum + int(deg[n]), t))
    KU = max(1, int((tile_deg.max() + P - 1) // P))

    # --- permuted node features and host-side q = x @ Wq^T / sqrt(DH) ---
    s = 1.0 / math.sqrt(DH)
    xf = np.asarray(x, dtype=np.float32)
    q_full = (xf @ np.asarray(Wq, dtype=np.float32).T) * s
    x_perm = np.zeros((NT * P, D), dtype=np.float16)
    x_perm[node_slot] = xf.astype(np.float16)
    q_perm = np.zeros((NT * P, D), dtype=np.float16)
    q_perm[node_slot] = q_full.astype(np.float16)
    x_permT = x_perm.T.copy()

    # --- per-tile edge lists (sorted by src slot for locality) ---
    src_slot = node_slot[src]
    dst_slot = node_slot[dst]
    et = dst_slot // P
    key = et * (1 << 32) + src_slot
    eorder = np.argsort(key, kind='stable')
    et_s = et[eorder]
    src_s = src_slot[eorder]
    dst_s = dst_slot[eorder]
    dloc_s = dst_s - et_s * P

    ecnt = np.bincount(et, minlength=NT)
    eoff = np.zeros(NT + 1, dtype=np.int64)
    np.cumsum(ecnt, out=eoff[1:])
    pos = np.arange(E) - eoff[et_s]

    # padded per-tile edge arrays: slot (tile, chunk j, part e) = edge j*128+e
    src_pad = np.zeros(NT * KU * P, dtype=np.int64)
    dst_pad = np.zeros(NT * KU * P, dtype=np.int64)
    dl_pad = np.full(NT * KU * P, 255, dtype=np.int64)
    flat = et_s * (KU * P) + pos
    src_pad[flat] = src_s
    dst_pad[flat] = dst_s
    dl_pad[flat] = dloc_s

    # x/q in edge order, transposed, tile-major so each tile's stream is one
    # contiguous HBM region: [NT, D, KU*P]
    x_edgeT = np.ascontiguousarray(
        x_perm[src_pad].T.reshape(D, NT, KU * P).transpose(1, 0, 2))
    q_edgeT = np.ascontiguousarray(
        q_perm[dst_pad].T.reshape(D, NT, KU * P).transpose(1, 0, 2))

    # compact dst-local indices: [P(e), NT*KU], f16 (0..127 or 255 pad)
    dl_all = dl_pad.reshape(NT, KU, P).transpose(2, 0, 1).astype(np.float16)

    # device consts
    iota_rep = np.tile(np.repeat(np.arange(P), KU).astype(np.float16),
                       (P, 1))                      # [P, P*KU], val[n*KU+j]=n
    blockind = (np.arange(D)[:, None] // DH
                == np.arange(H)[None, :]).astype(np.float16)   # [D, H]

    wkT = np.asarray(Wk, dtype=np.float32).T.astype(np.float16).copy()
    wvT = np.asarray(Wv, dtype=np.float32).T.astype(np.float16).copy()
    wsT = np.asarray(Wskip, dtype=np.float32).T.astype(np.float16).copy()
    for b in (bq, bk, bv, bskip):
        assert np.abs(np.asarray(b)).max() == 0.0, "nonzero biases not supported"

    in_maps = []
    for c in range(NCORES):
        t0, t1 = c * TPC, (c + 1) * TPC
        in_maps.append({
            "x_loc": x_perm[t0 * P:t1 * P].copy(),
            "x_locT": x_permT[:, t0 * P:t1 * P].copy(),
            "x_edgeT": x_edgeT[t0:t1].copy(),
            "q_edgeT": q_edgeT[t0:t1].copy(),
            "dl": dl_all[:, t0:t1, :].reshape(P, TPC * KU).copy(),
            "iota_rep": iota_rep, "blockind": blockind,
            "wkT": wkT, "wvT": wvT, "wsT": wsT,
        })
    return dict(N=N, E=E, TPC=TPC, NT=NT, KU=KU,
                node_slot=node_slot, in_maps=in_maps)


def _build(TPC, NT, KU):
    import concourse.bass as bass
    import concourse.bacc as bacc
    import concourse.mybir as mybir
    import concourse.tile as tile

    f16 = mybir.dt.float16
    f32 = mybir.dt.float32
    MUL = mybir.AluOpType.mult
    ADD = mybir.AluOpType.add
    ISEQ = mybir.AluOpType.is_equal
    EXP = mybir.ActivationFunctionType.Exp
    COPY = mybir.ActivationFunctionType.Copy
    AXX = mybir.AxisListType.X

    nc = bacc.Bacc("TRN2", target_bir_lowering=False, debug=False)
    x_loc = nc.dram_tensor("x_loc", [TPC * P, D], f16, kind="ExternalInput")
    x_locT = nc.dram_tensor("x_locT", [D, TPC * P], f16, kind="ExternalInput")
    x_edgeT = nc.dram_tensor("x_edgeT", [TPC, D, KU * P], f16,
                             kind="ExternalInput")
    q_edgeT = nc.dram_tensor("q_edgeT", [TPC, D, KU * P], f16,
                             kind="ExternalInput")
    dl_in = nc.dram_tensor("dl", [P, TPC * KU], f16, kind="ExternalInput")
    iota_in = nc.dram_tensor("iota_rep", [P, P * KU], f16, kind="ExternalInput")
    bi_in = nc.dram_tensor("blockind", [D, H], f16, kind="ExternalInput")
    wkT = nc.dram_tensor("wkT", [D, D], f16, kind="ExternalInput")
    wvT = nc.dram_tensor("wvT", [D, D], f16, kind="ExternalInput")
    wsT = nc.dram_tensor("wsT", [D, D], f16, kind="ExternalInput")
    out_t = nc.dram_tensor("out", [TPC * P, D], f32, kind="ExternalOutput")

    NB = 4
    groups = [(g * NBG, min(NBG, KU - g * NBG)) for g in range((KU + NBG - 1) // NBG)]

    with tile.TileContext(nc) as tc:
        with (
            tc.tile_pool(name="const", bufs=1) as cp,
            tc.tile_pool(name="sbuf", bufs=3) as sb,
            tc.tile_pool(name="big", bufs=3) as bigp,
            tc.tile_pool(name="psK", bufs=2, space="PSUM") as pk,
            tc.tile_pool(name="psV", bufs=2, space="PSUM") as pv,
            tc.tile_pool(name="psAl", bufs=2, space="PSUM") as pal,
            tc.tile_pool(name="psS", bufs=2, space="PSUM") as ps,
        ):
            wk_sb = cp.tile([D, D], f16, tag="wk")
            wv_sb = cp.tile([D, D], f16, tag="wv")
            ws_sb = cp.tile([D, D], f16, tag="ws")
            bi_sb = cp.tile([D, H], f16, tag="bi")
            iota_sb = cp.tile([P, P * KU], f16, tag="iota")
            dl_sb = cp.tile([P, TPC * KU], f16, tag="dl")
            s_loc = cp.tile([P, TPC * D], f16, tag="sloc")
            nc.sync.dma_start(out=wk_sb[:], in_=wkT[:])
            nc.sync.dma_start(out=wv_sb[:], in_=wvT[:])
            nc.sync.dma_start(out=ws_sb[:], in_=wsT[:])
            nc.sync.dma_start(out=bi_sb[:], in_=bi_in[:])
            nc.sync.dma_start(out=iota_sb[:], in_=iota_in[:])
            nc.sync.dma_start(out=dl_sb[:], in_=dl_in[:])

            # ---------------- local phase: skip = x@Wskip^T + x ----------------
            u = 0
            while u < TPC:
                lb = min(NB, TPC - u)
                xTl = sb.tile([P, NB * P], f16, tag="xT")
                nc.sync.dma_start(
                    out=xTl[:, :lb * P], in_=x_locT[:, u * P:(u + lb) * P])
                pq = pk.tile([P, NB * P], f32, tag="k")
                for b in range(lb):
                    nc.tensor.matmul(pq[:, b * P:(b + 1) * P],
                                     lhsT=xTl[:, b * P:(b + 1) * P], rhs=ws_sb[:],
                                     start=True, stop=True)
                xl = sb.tile([P, NB, P], f16, tag="xl")
                nc.sync.dma_start(
                    out=xl[:, :lb, :],
                    in_=x_loc[u * P:(u + lb) * P, :].rearrange(
                        "(b p) c -> p b c", p=P))
                nc.vector.tensor_tensor(
                    out=s_loc[:, u * D:(u + lb) * D].rearrange(
                        "p (b c) -> p b c", c=P),
                    in0=pq[:, :lb * P].rearrange("p (b c) -> p b c", c=P),
                    in1=xl[:, :lb, :], op=ADD)
                u += lb

            # ---------------- edge phase ----------------
            for u in range(TPC):
                xeT = bigp.tile([P, KU * P], f16, tag="xeT")
                nc.sync.dma_start(out=xeT[:], in_=x_edgeT[u, :, :])
                qeT = bigp.tile([P, KU * P], f16, tag="qeT")
                nc.sync.dma_start(out=qeT[:], in_=q_edgeT[u, :, :])
                # scatter one-hot, interleaved layout [e, (n,KU)]:
                # ohT[e, n*KU+j] = (dl[e, u*KU+j] == n)
                ohT = bigp.tile([P, P * KU], f16, tag="ohT")
                nc.vector.tensor_tensor(
                    out=ohT[:].rearrange("p (n j) -> p n j", j=KU),
                    in0=iota_sb[:].rearrange("p (n j) -> p n j", j=KU),
                    in1=dl_sb[:, None, u * KU:(u + 1) * KU].to_broadcast(
                        [P, P, KU]),
                    op=ISEQ)
                ohT3 = ohT[:].rearrange("p (n j) -> p n j", j=KU)

                psS = ps.tile([P, 132], f32, tag="acc")
                for gi, (j0, gsz) in enumerate(groups):
                    psK = pk.tile([P, NB * P], f32, tag="k")
                    psV = pv.tile([P, NBG, P], f32, tag="v")
                    for cc in range(gsz):
                        j = j0 + cc
                        nc.tensor.matmul(
                            psK[:, cc * P:(cc + 1) * P],
                            lhsT=wk_sb[:], rhs=xeT[:, j * P:(j + 1) * P],
                            start=True, stop=True)
                    for cc in range(gsz):
                        j = j0 + cc
                        nc.tensor.matmul(
                            psV[:, cc, :],
                            lhsT=xeT[:, j * P:(j + 1) * P], rhs=wv_sb[:],
                            start=True, stop=True)
                    # kT PSUM->SBUF f16 on Act so the qk multiply runs 2x
                    ksb = sb.tile([P, NBG * P], f16, tag="ksb")
                    nc.scalar.activation(out=ksb[:, :gsz * P],
                                         in_=psK[:, :gsz * P], func=COPY)
                    qk = sb.tile([P, NBG * P], f16, tag="qk")
                    qk_eng = nc.vector if (gi == 3 and u % 2 == 0) else nc.gpsimd
                    qk_eng.tensor_tensor(
                        out=qk[:, :gsz * P],
                        in0=qeT[:, j0 * P:j0 * P + gsz * P],
                        in1=ksb[:, :gsz * P], op=MUL)
                    # per-head reduction on PE: alpha[e,h] = sum_c qk[c,e]*bi[c,h]
                    psA = pal.tile([P, NBG * H], f32, tag="al")
                    for cc in range(gsz):
                        nc.tensor.matmul(
                            psA[:, cc * H:(cc + 1) * H],
                            lhsT=qk[:, cc * P:(cc + 1) * P], rhs=bi_sb[:],
                            start=True, stop=True)
                    X = sb.tile([P, NBG, 132], f16, tag="X")
                    nc.scalar.activation(
                        out=X[:, :gsz, 128:132],
                        in_=psA[:, :gsz * H].rearrange("p (a h) -> p a h", h=H),
                        func=EXP)
                    nc.vector.tensor_tensor(
                        out=X[:, :gsz, 0:128].rearrange(
                            "p a (h e) -> p a h e", e=DH),
                        in0=psV[:, :gsz, :].rearrange(
                            "p a (h e) -> p a h e", e=DH),
                        in1=X[:, :gsz, 128:132, None].to_broadcast(
                            [P, gsz, H, DH]),
                        op=MUL)
                    for cc in range(gsz):
                        j = j0 + cc
                        nc.tensor.matmul(
                            psS[:, 0:132],
                            lhsT=ohT3[:, :, j],
                            rhs=X[:, cc, 0:132],
                            start=(j == 0), stop=(j == KU - 1))

                dn = sb.tile([P, H], f32, tag="dn")
                nc.vector.tensor_scalar(out=dn[:], in0=psS[:, 128:132],
                                        scalar1=1e-16, scalar2=None, op0=ADD)
                rc = sb.tile([P, H], f32, tag="rc")
                nc.vector.reciprocal(out=rc[:], in_=dn[:])
                ot = sb.tile([P, D], f32, tag="ot")
                for h in range(H):
                    nc.scalar.activation(
                        out=ot[:, h * DH:(h + 1) * DH],
                        in_=psS[:, h * DH:(h + 1) * DH],
                        func=COPY, scale=rc[:, h:h + 1])
                of = sb.tile([P, D], f32, tag="of")
                nc.gpsimd.tensor_tensor(
                    out=of[:], in0=ot[:], in1=s_loc[:, u * D:(u + 1) * D], op=ADD)
                nc.sync.dma_start(out=out_t[u * P:(u + 1) * P, :], in_=of[:])

    nc.finalize()
    return nc


def _run(inputs, trace=False):
    _ensure_hooks()
    from concourse.bass_utils import run_bass_kernel_spmd

    meta = _prep(**inputs)
    key = (meta['TPC'], meta['NT'], meta['KU'])
    if key not in _prog_cache:
        _prog_cache[key] = _build(*key)
    nc = _prog_cache[key]
    res = run_bass_kernel_spmd(nc, meta['in_maps'],
                               core_ids=list(range(NCORES)), trace=trace)
    outs = [res.results[c]["out"] for c in range(NCORES)]
    out_perm = np.concatenate(outs, axis=0)
    out = out_perm[meta['node_slot']].astype(np.float32)
    return out, res


def kernel(**inputs) -> np.ndarray:
    out, _ = _run(inputs, trace=False)
    return out
